# revision 11
# baseline (speedup 1.0000x reference)
"""Additive attention (Bahdanau) kernel for 8 Trainium2 NeuronCores.

Reference computation (per batch b):
    h   = enc_seq @ W_h.T                 [T, H]
    s   = dec_state @ W_s.T               [H]
    e_t = v . tanh(h_t + s)               [T]
    e   = where(mask==0, -1e9, e)
    a   = softmax(e)
    ctx = sum_t a_t * enc_seq[t]          [H]

Sharding: data-parallel over batch B=32 -> 4 batches per core, weights
replicated.

Key optimizations over the naive layout:
  * Mask compaction on the host: positions with mask==0 have softmax
    weight exactly 0 (exp(-1e9) underflows), so only the unmasked
    positions are shipped/computed.  All batches are padded to the same
    L = ceil(max_count/256)*256; padding columns get enc=0 and a -1e9
    additive bias, contributing exactly 0, so the result is identical.
  * Chunk-grouped schedule: each 512-wide t-chunk is processed for all
    4 batches together, so the per-batch e-rows share one PSUM tile
    (partitions 0/32/64/96) and the mask-add / exp / broadcast run once
    per group instead of once per (batch, chunk).
  * The e = v . tanh dot uses a [128, 32] stationary with v replicated
    32x, writing 32 identical PSUM partitions per batch: matmul cost is
    column-bound so the extra rows are free, and every partition of the
    PSUM tile is initialized (no junk for the batched mask-add / exp).
  * One enc DMA per chunk group (all 4 batches packed contiguously on
    the host, 16KB per partition line), one strided-partition DMA out
    and one broadcast DMA back for the softmax-row bounce: 3 DMA
    dispatches per group instead of 12.
  * s = dec @ W_s.T is computed on the host (it is tiny) and shipped
    as a [128, OT, BL] f32 bias table.
  * Latency-critical bounce DMAs ride the Scalar engine's hardware DGE
    ring and constants ride the Tensor ring, so they never queue behind
    the bulk enc transfers on the Sync ring.
"""

import os
import sys
import numpy as np

sys.path.insert(0, "/opt/trn_rl_repo")

import ml_dtypes

B, T, H = 32, 4096, 512
NCORES = 8
BL = B // NCORES          # 4 batches per core
P = 128
KT = H // P               # 4 contraction tiles
OT = H // P               # 4 output tiles
NEG = -1.0e9

_CACHE = {}


def _chunk_widths(L):
    ws = [512] * (L // 512)
    if L % 512:
        ws.append(L % 512)
    return ws


def _build(L):
    import concourse.bass as bass
    import concourse.tile as tile
    from concourse import bacc, mybir
    from contextlib import ExitStack

    f32 = mybir.dt.float32
    bf16 = mybir.dt.bfloat16
    ts = bass.ts
    Alu = mybir.AluOpType
    Act = mybir.ActivationFunctionType

    widths = _chunk_widths(L)
    NG = len(widths)
    offs = [BL * 4 * sum(widths[:i]) for i in range(NG)]  # into [128, BL*4L]
    toffs = [sum(widths[:i]) for i in range(NG)]          # into [*, L]

    nc = bacc.Bacc()

    enc_p = nc.declare_dram_parameter("enc_p", [P, BL * 4 * L], bf16, isOutput=False)
    maskb = nc.declare_dram_parameter("maskb", [BL, L], bf16, isOutput=False)
    s_in = nc.declare_dram_parameter("s_in", [P, OT, BL], f32, isOutput=False)
    w_ht = nc.declare_dram_parameter("w_ht", [H, H], bf16, isOutput=False)
    v_in = nc.declare_dram_parameter("v_in", [P, KT, 32], bf16, isOutput=False)
    out_e = nc.declare_dram_parameter("out", [P, BL, OT], f32, isOutput=True)
    sums_e = nc.declare_dram_parameter("sums_out", [P, 1], f32, isOutput=True)

    with tile.TileContext(nc) as tc, ExitStack() as ctx:
        const = ctx.enter_context(tc.tile_pool(name="const", bufs=1))
        encp = ctx.enter_context(tc.tile_pool(name="encp", bufs=3))
        tanhp = ctx.enter_context(tc.tile_pool(name="tanhp", bufs=8))
        toutp = ctx.enter_context(tc.tile_pool(name="toutp", bufs=3))
        erowp = ctx.enter_context(tc.tile_pool(name="erowp", bufs=3))
        pexp = ctx.enter_context(tc.tile_pool(name="pexp", bufs=3))
        pbcp = ctx.enter_context(tc.tile_pool(name="pbcp", bufs=3))
        ctxp = ctx.enter_context(tc.tile_pool(name="ctxp", bufs=4))
        dramp = ctx.enter_context(tc.tile_pool(name="dramp", bufs=3, space="DRAM"))
        php = ctx.enter_context(tc.tile_pool(name="php", bufs=6, space="PSUM"))
        pep = ctx.enter_context(tc.tile_pool(name="pep", bufs=2, space="PSUM"))

        # ---- constants on the tensor DMA ring, enc bulk on the sync ring ----
        def fetch_group(g, w):
            et = encp.tile([P, BL, KT, 512], bf16, tag="enc_tile", name=f"et{g}")
            src = enc_p[:, offs[g] : offs[g] + BL * 4 * w].rearrange(
                "p (b k t) -> p b k t", b=BL, k=KT
            )
            for b in range(BL):
                nc.sync.dma_start(et[:, b, :, :w], src[:, b, :, :])
            return et

        w_sb = const.tile([P, KT, H], bf16, tag="w_sb")
        nc.scalar.dma_start(w_sb[:], w_ht.rearrange("(k p) o -> p k o", p=P))
        et_next = fetch_group(0, widths[0])
        v_sb = const.tile([P, KT, 32], bf16, tag="v_sb")
        nc.scalar.dma_start(v_sb[:], v_in[:, :, :])
        s_sb = const.tile([P, OT, BL], f32, tag="s_sb")
        nc.scalar.dma_start(s_sb[:], s_in[:, :, :])
        mask_sb = const.tile([P, L], bf16, tag="mask_sb")
        for b in range(BL):
            nc.scalar.dma_start(
                mask_sb[32 * b : 32 * b + 32, :],
                maskb[b : b + 1, :].to_broadcast((32, L)),
            )

        sums = const.tile([P, NG + 1], f32, tag="sums")
        out_sb = const.tile([P, BL, OT], f32, tag="out_sb")

        # ---- context accumulators (per batch) ----
        cas = []
        for b in range(BL):
            ca = ctxp.tile([P, OT, NG], f32, tag=f"ca{b}", name=f"ca{b}")
            cas.append(ca)

        # ---- main pipeline over chunk groups ----
        for g, w in enumerate(widths):
            et = et_next
            if g + 1 < NG:
                et_next = fetch_group(g + 1, widths[g + 1])

            pe_t = pep.tile([P, 512], f32, tag="pe")
            for o in range(OT):
                tts = []
                for b in range(BL):
                    ph = php.tile([P, 512], f32, tag="ph")
                    for k in range(KT):
                        nc.tensor.matmul(
                            ph[:, :w],
                            w_sb[:, k, ts(o, P)],
                            et[:, b, k, :w],
                            start=(k == 0),
                            stop=(k == KT - 1),
                        )
                    tt = tanhp.tile([P, 512], bf16, tag="tt")
                    nc.scalar.activation(
                        tt[:, :w], ph[:, :w], Act.Tanh, bias=s_sb[:, o, b : b + 1]
                    )
                    tts.append(tt)
                for b in range(BL):
                    nc.tensor.matmul(
                        pe_t[32 * b : 32 * b + 32, :w],
                        v_sb[:, o, :],
                        tts[b][:, :w],
                        start=(o == 0),
                        stop=(o == OT - 1),
                        tile_position=(0, 32 * b),
                        skip_group_check=True,
                    )

            # e = pe + maskbias (batched over the 4 batches' row groups)
            erow = erowp.tile([P, 512], f32, tag="erow")
            nc.vector.tensor_add(
                erow[:, :w], pe_t[:, :w], mask_sb[:, toffs[g] : toffs[g] + w]
            )
            # p = exp(e) unnormalized + per-partition chunk sums
            pex = pexp.tile([P, 512], bf16, tag="pex")
            nc.scalar.activation(
                pex[:, :w], erow[:, :w], Act.Exp, accum_out=sums[:, g : g + 1]
            )
            # broadcast the 4 p-rows to all 128 partitions via a DRAM bounce
            pd = dramp.tile([1, BL, 512], bf16, tag="pd")
            nc.scalar.dma_start(pd[0, :, :w], pex[0:128:32, :w])
            pb = pbcp.tile([P, BL, 512], bf16, tag="pb")
            nc.scalar.dma_start(
                pb[:, :, :w], pd[:, :, :w].to_broadcast((P, BL, w))
            )
            # ctx_raw[:, ht] += sum_t p[t] * x[t]
            for b in range(BL):
                for ht in range(KT):
                    to = toutp.tile([P, 512], bf16, tag="to", name="to")
                    nc.vector.scalar_tensor_tensor(
                        out=to[:, :w],
                        in0=et[:, b, ht, :w],
                        scalar=1.0,
                        in1=pb[:, b, :w],
                        op0=Alu.mult,
                        op1=Alu.mult,
                        accum_out=cas[b][:, ht, g : g + 1],
                    )

        # ---- tails: ship raw context + softmax sums; host divides ----
        nc.vector.tensor_reduce(
            sums[:, NG : NG + 1], sums[:, 0:NG], axis=mybir.AxisListType.X, op=Alu.add
        )
        nc.scalar.dma_start(sums_e[:, :], sums[:, NG : NG + 1])
        for b in range(BL):
            nc.vector.tensor_reduce(
                out_sb[:, b, :], cas[b][:], axis=mybir.AxisListType.X, op=Alu.add
            )
        nc.scalar.dma_start(out_e[:, :, :], out_sb[:, :, :])

    nc.finalize()
    return nc


def _prep_in_maps(enc_seq, enc_mask, dec_state, W_h, W_s, v):
    bf = ml_dtypes.bfloat16
    w_ht = np.ascontiguousarray(W_h.T).astype(bf)
    v_rep = np.ascontiguousarray(
        np.broadcast_to(v.reshape(KT, P).T[:, :, None], (P, KT, 32))
    ).astype(bf)
    s_all = dec_state.astype(np.float32) @ W_s.astype(np.float32).T  # [B, H]

    cnts = (enc_mask != 0).sum(axis=1)
    L = max(256, int(-(-int(cnts.max()) // 256) * 256))
    widths = _chunk_widths(L)

    in_maps = []
    for c in range(NCORES):
        sl = slice(c * BL, (c + 1) * BL)
        enc_p = np.zeros((P, BL * 4 * L), dtype=bf)
        maskb = np.full((BL, L), np.float32(NEG), dtype=bf)
        off = 0
        t0 = 0
        for w in widths:
            blk = np.zeros((P, BL, KT, w), dtype=bf)
            for bi, bg in enumerate(range(c * BL, (c + 1) * BL)):
                idx = np.flatnonzero(enc_mask[bg] != 0)
                n = idx.size
                lo, hi = t0, min(t0 + w, n)
                if hi > lo:
                    xg = enc_seq[bg][idx[lo:hi]]            # [hi-lo, H]
                    blk[:, bi, :, : hi - lo] = (
                        xg.T.reshape(KT, P, hi - lo).transpose(1, 0, 2).astype(bf)
                    )
            enc_p[:, off : off + BL * 4 * w] = blk.reshape(P, BL * 4 * w)
            off += BL * 4 * w
            t0 += w
        for bi, bg in enumerate(range(c * BL, (c + 1) * BL)):
            maskb[bi, : int(cnts[bg])] = 0.0
        # s table: s_in[p, o, b] = s[b, o*128+p]
        s_in = np.ascontiguousarray(
            s_all[sl].T.reshape(OT, P, BL).transpose(1, 0, 2)
        ).astype(np.float32)
        in_maps.append(
            {
                "enc_p": enc_p,
                "maskb": maskb,
                "s_in": s_in,
                "w_ht": w_ht,
                "v_in": v_rep,
            }
        )
    return in_maps, L


def _run(inputs, trace=False):
    from concourse.bass_utils import run_bass_kernel_spmd

    in_maps, L = _prep_in_maps(**{k: np.asarray(v) for k, v in inputs.items()})
    if L not in _CACHE:
        _CACHE[L] = _build(L)
    nc = _CACHE[L]
    res = run_bass_kernel_spmd(nc, in_maps, core_ids=list(range(NCORES)), trace=trace)
    outs = []
    for c in range(NCORES):
        o = np.asarray(res.results[c]["out"], dtype=np.float32)  # [P, BL, OT]
        sm = np.asarray(res.results[c]["sums_out"], dtype=np.float32)  # [P, 1]
        ctx_raw = o.transpose(1, 2, 0).reshape(BL, H)
        denom = sm[32 * np.arange(BL), 0][:, None]
        outs.append(ctx_raw / denom)
    return np.concatenate(outs, axis=0).astype(np.float32), res


def kernel(**inputs):
    out, _ = _run(inputs, trace=False)
    return out


# revision 13
# speedup vs baseline: 1.2407x; 1.2407x over previous
"""Additive attention (Bahdanau) kernel for 8 Trainium2 NeuronCores.

Reference computation (per batch b):
    h   = enc_seq @ W_h.T                 [T, H]
    s   = dec_state @ W_s.T               [H]
    e_t = v . tanh(h_t + s)               [T]
    e   = where(mask==0, -1e9, e)
    a   = softmax(e)
    ctx = sum_t a_t * enc_seq[t]          [H]

Sharding: data-parallel over batch B=32 -> 4 batches per core, weights
replicated.

Key optimizations over the naive layout:
  * Mask compaction on the host: positions with mask==0 have softmax
    weight exactly 0 (exp(-1e9) underflows), so only the unmasked
    positions are shipped/computed.  All batches are padded to the same
    L = ceil(max_count/256)*256; padding columns get enc=0 and a -1e9
    additive bias, contributing exactly 0, so the result is identical.
  * Chunk-grouped schedule: each 512-wide t-chunk is processed for all
    4 batches together, so the per-batch e-rows share one PSUM tile
    (partitions 0/32/64/96) and the mask-add / exp / broadcast run once
    per group instead of once per (batch, chunk).
  * The e = v . tanh dot uses a [128, 32] stationary with v replicated
    32x, writing 32 identical PSUM partitions per batch: matmul cost is
    column-bound so the extra rows are free, and every partition of the
    PSUM tile is initialized (no junk for the batched mask-add / exp).
  * One enc DMA per chunk group (all 4 batches packed contiguously on
    the host, 16KB per partition line), one strided-partition DMA out
    and one broadcast DMA back for the softmax-row bounce: 3 DMA
    dispatches per group instead of 12.
  * s = dec @ W_s.T is computed on the host (it is tiny) and shipped
    as a [128, OT, BL] f32 bias table.
  * Latency-critical bounce DMAs ride the Scalar engine's hardware DGE
    ring and constants ride the Tensor ring, so they never queue behind
    the bulk enc transfers on the Sync ring.
"""

import os
import sys
import numpy as np

sys.path.insert(0, "/opt/trn_rl_repo")

import ml_dtypes

B, T, H = 32, 4096, 512
NCORES = 8
BL = B // NCORES          # 4 batches per core
P = 128
KT = H // P               # 4 contraction tiles
OT = H // P               # 4 output tiles
NEG = -1.0e9

_CACHE = {}


def _chunk_widths(L):
    ws = [512] * (L // 512)
    if L % 512:
        ws.append(L % 512)
    return ws


def _build(L):
    import concourse.bass as bass
    import concourse.tile as tile
    from concourse import bacc, library_config, mybir
    from contextlib import ExitStack

    f32 = mybir.dt.float32
    bf16 = mybir.dt.bfloat16
    ts = bass.ts
    Alu = mybir.AluOpType
    Act = mybir.ActivationFunctionType

    widths = _chunk_widths(L)
    NG = len(widths)
    offs = [BL * 4 * sum(widths[:i]) for i in range(NG)]  # into [128, BL*4L]
    toffs = [sum(widths[:i]) for i in range(NG)]          # into [*, L]

    nc = bacc.Bacc()

    enc_p = nc.declare_dram_parameter("enc_p", [P, BL * 4 * L], bf16, isOutput=False)
    maskb = nc.declare_dram_parameter("maskb", [BL, L], bf16, isOutput=False)
    s_in = nc.declare_dram_parameter("s_in", [P, OT, BL], f32, isOutput=False)
    w_ht = nc.declare_dram_parameter("w_ht", [H, H], bf16, isOutput=False)
    v_in = nc.declare_dram_parameter("v_in", [P, KT, 32], bf16, isOutput=False)
    out_e = nc.declare_dram_parameter("out", [P, BL, OT], f32, isOutput=True)
    sums_e = nc.declare_dram_parameter("sums_out", [P, 1], f32, isOutput=True)

    with tile.TileContext(nc) as tc, ExitStack() as ctx:
        nc.gpsimd.load_library(library_config.attn)
        const = ctx.enter_context(tc.tile_pool(name="const", bufs=1))
        encp = ctx.enter_context(tc.tile_pool(name="encp", bufs=3))
        tanhp = ctx.enter_context(tc.tile_pool(name="tanhp", bufs=8))
        toutp = ctx.enter_context(tc.tile_pool(name="toutp", bufs=3))
        erowp = ctx.enter_context(tc.tile_pool(name="erowp", bufs=3))
        pexp = ctx.enter_context(tc.tile_pool(name="pexp", bufs=3))
        pbcp = ctx.enter_context(tc.tile_pool(name="pbcp", bufs=3))
        ctxp = ctx.enter_context(tc.tile_pool(name="ctxp", bufs=4))
        php = ctx.enter_context(tc.tile_pool(name="php", bufs=6, space="PSUM"))
        pep = ctx.enter_context(tc.tile_pool(name="pep", bufs=2, space="PSUM"))

        # ---- constants on the tensor DMA ring, enc bulk on the sync ring ----
        def fetch_group(g, w):
            et = encp.tile([P, BL, KT, 512], bf16, tag="enc_tile", name=f"et{g}")
            src = enc_p[:, offs[g] : offs[g] + BL * 4 * w].rearrange(
                "p (b k t) -> p b k t", b=BL, k=KT
            )
            for b in range(BL):
                nc.sync.dma_start(et[:, b, :, :w], src[:, b, :, :])
            return et

        w_sb = const.tile([P, KT, H], bf16, tag="w_sb")
        nc.scalar.dma_start(w_sb[:], w_ht.rearrange("(k p) o -> p k o", p=P))
        et_next = fetch_group(0, widths[0])
        v_sb = const.tile([P, KT, 32], bf16, tag="v_sb")
        nc.scalar.dma_start(v_sb[:], v_in[:, :, :])
        s_sb = const.tile([P, OT, BL], f32, tag="s_sb")
        nc.scalar.dma_start(s_sb[:], s_in[:, :, :])
        mask_sb = const.tile([P, L], bf16, tag="mask_sb")
        for b in range(BL):
            nc.scalar.dma_start(
                mask_sb[32 * b : 32 * b + 32, :],
                maskb[b : b + 1, :].to_broadcast((32, L)),
            )

        sums = const.tile([P, NG + 1], f32, tag="sums")
        out_sb = const.tile([P, BL, OT], f32, tag="out_sb")

        # ---- context accumulators (per batch) ----
        cas = []
        for b in range(BL):
            ca = ctxp.tile([P, OT, NG], f32, tag=f"ca{b}", name=f"ca{b}")
            cas.append(ca)

        # ---- main pipeline over chunk groups ----
        for g, w in enumerate(widths):
            et = et_next
            if g + 1 < NG:
                et_next = fetch_group(g + 1, widths[g + 1])

            pe_t = pep.tile([P, 512], f32, tag="pe")
            for o in range(OT):
                tts = []
                for b in range(BL):
                    ph = php.tile([P, 512], f32, tag="ph")
                    for k in range(KT):
                        nc.tensor.matmul(
                            ph[:, :w],
                            w_sb[:, k, ts(o, P)],
                            et[:, b, k, :w],
                            start=(k == 0),
                            stop=(k == KT - 1),
                        )
                    tt = tanhp.tile([P, 512], bf16, tag="tt")
                    nc.scalar.activation(
                        tt[:, :w], ph[:, :w], Act.Tanh, bias=s_sb[:, o, b : b + 1]
                    )
                    tts.append(tt)
                for b in range(BL):
                    nc.tensor.matmul(
                        pe_t[32 * b : 32 * b + 32, :w],
                        v_sb[:, o, :],
                        tts[b][:, :w],
                        start=(o == 0),
                        stop=(o == OT - 1),
                        tile_position=(0, 32 * b),
                        skip_group_check=True,
                    )

            # e = pe + maskbias (batched over the 4 batches' row groups)
            erow = erowp.tile([P, 512], f32, tag="erow")
            nc.vector.tensor_add(
                erow[:, :w], pe_t[:, :w], mask_sb[:, toffs[g] : toffs[g] + w]
            )
            # p = exp(e) unnormalized + per-partition chunk sums
            pex = pexp.tile([P, 512], bf16, tag="pex")
            nc.scalar.activation(
                pex[:, :w], erow[:, :w], Act.Exp, accum_out=sums[:, g : g + 1]
            )
            # hop the 4 p-rows to partition 0 (tiny SBUF->SBUF DMAs), then
            # broadcast to all 128 partitions on GPSIMD (no DRAM bounce)
            tmp0 = pexp.tile([1, BL, 512], bf16, tag="tmp0", name="tmp0")
            for b in range(BL):
                nc.scalar.dma_start(
                    tmp0[0:1, b, :w], pex[32 * b : 32 * b + 1, :w]
                )
            pb = pbcp.tile([P, BL, 512], bf16, tag="pb")
            for b in range(BL):
                nc.gpsimd.partition_broadcast(
                    pb[:, b, :w], tmp0[0:1, b, :w]
                )
            # ctx_raw[:, ht] += sum_t p[t] * x[t]
            for b in range(BL):
                for ht in range(KT):
                    to = toutp.tile([P, 512], bf16, tag="to", name="to")
                    nc.vector.scalar_tensor_tensor(
                        out=to[:, :w],
                        in0=et[:, b, ht, :w],
                        scalar=1.0,
                        in1=pb[:, b, :w],
                        op0=Alu.mult,
                        op1=Alu.mult,
                        accum_out=cas[b][:, ht, g : g + 1],
                    )

        # ---- tails: ship raw context + softmax sums; host divides ----
        nc.vector.tensor_reduce(
            sums[:, NG : NG + 1], sums[:, 0:NG], axis=mybir.AxisListType.X, op=Alu.add
        )
        nc.scalar.dma_start(sums_e[:, :], sums[:, NG : NG + 1])
        for b in range(BL):
            nc.vector.tensor_reduce(
                out_sb[:, b, :], cas[b][:], axis=mybir.AxisListType.X, op=Alu.add
            )
        nc.scalar.dma_start(out_e[:, :, :], out_sb[:, :, :])

    nc.finalize()
    return nc


def _prep_in_maps(enc_seq, enc_mask, dec_state, W_h, W_s, v):
    bf = ml_dtypes.bfloat16
    w_ht = np.ascontiguousarray(W_h.T).astype(bf)
    v_rep = np.ascontiguousarray(
        np.broadcast_to(v.reshape(KT, P).T[:, :, None], (P, KT, 32))
    ).astype(bf)
    s_all = dec_state.astype(np.float32) @ W_s.astype(np.float32).T  # [B, H]

    cnts = (enc_mask != 0).sum(axis=1)
    L = max(256, int(-(-int(cnts.max()) // 256) * 256))
    widths = _chunk_widths(L)

    in_maps = []
    for c in range(NCORES):
        sl = slice(c * BL, (c + 1) * BL)
        enc_p = np.zeros((P, BL * 4 * L), dtype=bf)
        maskb = np.full((BL, L), np.float32(NEG), dtype=bf)
        off = 0
        t0 = 0
        for w in widths:
            blk = np.zeros((P, BL, KT, w), dtype=bf)
            for bi, bg in enumerate(range(c * BL, (c + 1) * BL)):
                idx = np.flatnonzero(enc_mask[bg] != 0)
                n = idx.size
                lo, hi = t0, min(t0 + w, n)
                if hi > lo:
                    xg = enc_seq[bg][idx[lo:hi]]            # [hi-lo, H]
                    blk[:, bi, :, : hi - lo] = (
                        xg.T.reshape(KT, P, hi - lo).transpose(1, 0, 2).astype(bf)
                    )
            enc_p[:, off : off + BL * 4 * w] = blk.reshape(P, BL * 4 * w)
            off += BL * 4 * w
            t0 += w
        for bi, bg in enumerate(range(c * BL, (c + 1) * BL)):
            maskb[bi, : int(cnts[bg])] = 0.0
        # s table: s_in[p, o, b] = s[b, o*128+p]
        s_in = np.ascontiguousarray(
            s_all[sl].T.reshape(OT, P, BL).transpose(1, 0, 2)
        ).astype(np.float32)
        in_maps.append(
            {
                "enc_p": enc_p,
                "maskb": maskb,
                "s_in": s_in,
                "w_ht": w_ht,
                "v_in": v_rep,
            }
        )
    return in_maps, L


def _run(inputs, trace=False):
    from concourse.bass_utils import run_bass_kernel_spmd

    in_maps, L = _prep_in_maps(**{k: np.asarray(v) for k, v in inputs.items()})
    if L not in _CACHE:
        _CACHE[L] = _build(L)
    nc = _CACHE[L]
    res = run_bass_kernel_spmd(nc, in_maps, core_ids=list(range(NCORES)), trace=trace)
    outs = []
    for c in range(NCORES):
        o = np.asarray(res.results[c]["out"], dtype=np.float32)  # [P, BL, OT]
        sm = np.asarray(res.results[c]["sums_out"], dtype=np.float32)  # [P, 1]
        ctx_raw = o.transpose(1, 2, 0).reshape(BL, H)
        denom = sm[32 * np.arange(BL), 0][:, None]
        outs.append(ctx_raw / denom)
    return np.concatenate(outs, axis=0).astype(np.float32), res


def kernel(**inputs):
    out, _ = _run(inputs, trace=False)
    return out
